# revision 3
# baseline (speedup 1.0000x reference)
"""NeuronSAT GNN message passing on 8 Trainium2 NeuronCores.

Sharding: data-parallel over graphs — graph g lives entirely on core g.
All state (h, c), weights, and the per-graph bipartite incidence matrices
are SBUF-resident for all 26 rounds; HBM traffic is a one-time ~13MB load
per core plus a 4-byte result store.

Layout: feature-major [128=D, nodes]. Literal<->clause aggregations are
dense matmuls against the (zero-padded) incidence matrix A [800,1200]
(and its transpose), with the message-MLP's last-layer bias folded in
analytically (every clause has exactly K=5 edges; literal degrees give a
precomputed outer-product bias). The literal "flip" (negation) indexing
is a pure column-slice swap in this layout.

Matmuls run in float32r (single-pass fp32, ~1e-4 relative rounding).
"""

import sys

sys.path.insert(0, "/opt/trn_rl_repo")

import numpy as np

import concourse.bacc as bacc
import concourse.mybir as mybir
import concourse.tile as tile
from concourse.bass_utils import run_bass_kernel_spmd

# Problem dims (fixed by the reference).
NG = 8          # graphs == cores
NV = 400        # vars per graph
NCL = 1200      # clauses per graph
KLIT = 5        # literals per clause
NLIT = 2 * NV   # 800 literal nodes per graph
NNG = NLIT + NCL  # 2000 nodes per graph
D = 128
ROUNDS = 26
LCH = (NLIT + 127) // 128   # 7 literal 128-chunks (last has 32)
CCH = (NCL + 127) // 128    # 10 clause 128-chunks (last has 48)

F32 = mybir.dt.float32
F32R = mybir.dt.float32r
AF = mybir.ActivationFunctionType

LIT_COLS = [(0, 400), (400, 400)]            # literal col chunks
CL_COLS = [(0, 400), (400, 400), (800, 400)]  # clause col chunks


def _lchunk(i):
    return 128 if i < LCH - 1 else NLIT - 128 * (LCH - 1)


def _cchunk(j):
    return 128 if j < CCH - 1 else NCL - 128 * (CCH - 1)


def build_nc(rounds=ROUNDS, debug_state=False):
    nc = bacc.Bacc(None, target_bir_lowering=False)

    def din(name, shape, dt=F32R):
        return nc.declare_dram_parameter(name, list(shape), dt, isOutput=False)

    a_lc_d = din("a_lc", [128, LCH, NCL])
    a_cl_d = din("a_cl", [128, CCH, NLIT])
    h0_lit_d = din("h0_lit", [128, NLIT])
    h0_cl_d = din("h0_cl", [128, NCL])
    lmsgT_d = din("lmsgT", [128, 3, 128])
    lmsg_b_d = din("lmsg_b", [128, 2], F32)
    cmsgT_d = din("cmsgT", [128, 3, 128])
    cmsg_b_d = din("cmsg_b", [128, 2], F32)
    aggc_b_d = din("aggc_b", [128, 1], F32)
    aggl_b_d = din("aggl_b", [128, NLIT], F32)
    cu_wihT_d = din("cu_wihT", [128, 4, 128])
    cu_whhT_d = din("cu_whhT", [128, 4, 128])
    cu_b_d = din("cu_b", [128, 4], F32)
    lu_wihTa_d = din("lu_wihTa", [128, 4, 128])
    lu_wihTb_d = din("lu_wihTb", [128, 4, 128])
    lu_whhT_d = din("lu_whhT", [128, 4, 128])
    lu_b_d = din("lu_b", [128, 4], F32)
    vw0T_d = din("vw0T", [128, 128])
    vw1T_d = din("vw1T", [128, 128])
    vw2T_d = din("vw2T", [128, 1])
    vb_d = din("vb", [128, 2], F32)

    out_d = nc.declare_dram_parameter("out", [1, 1], F32, isOutput=True)
    if debug_state:
        dbg_hl = nc.declare_dram_parameter("dbg_hl", [128, NLIT], F32, isOutput=True)
        dbg_hc = nc.declare_dram_parameter("dbg_hc", [128, NCL], F32, isOutput=True)
        dbg_cl = nc.declare_dram_parameter("dbg_cl", [128, NLIT], F32, isOutput=True)
        dbg_cc = nc.declare_dram_parameter("dbg_cc", [128, NCL], F32, isOutput=True)

    with tile.TileContext(nc) as tc:
        with tc.tile_pool(name="singles", bufs=1) as singles, \
             tc.tile_pool(name="work", bufs=2) as work, \
             tc.tile_pool(name="ps", bufs=3, space="PSUM") as psp, \
             tc.tile_pool(name="psg", bufs=4, space="PSUM") as psg:

            def load(name, shape, dram, dt=F32R):
                t = singles.tile(list(shape), dt, tag=name)
                nc.sync.dma_start(out=t[:], in_=dram[:])
                return t

            a_lc = load("a_lc", [128, LCH, NCL], a_lc_d)
            a_cl = load("a_cl", [128, CCH, NLIT], a_cl_d)
            lmsgT = load("lmsgT", [128, 3, 128], lmsgT_d)
            lmsg_b = load("lmsg_b", [128, 2], lmsg_b_d, F32)
            cmsgT = load("cmsgT", [128, 3, 128], cmsgT_d)
            cmsg_b = load("cmsg_b", [128, 2], cmsg_b_d, F32)
            aggc_b = load("aggc_b", [128, 1], aggc_b_d, F32)
            aggl_b = load("aggl_b", [128, NLIT], aggl_b_d, F32)
            cu_wihT = load("cu_wihT", [128, 4, 128], cu_wihT_d)
            cu_whhT = load("cu_whhT", [128, 4, 128], cu_whhT_d)
            cu_b = load("cu_b", [128, 4], cu_b_d, F32)
            lu_wihTa = load("lu_wihTa", [128, 4, 128], lu_wihTa_d)
            lu_wihTb = load("lu_wihTb", [128, 4, 128], lu_wihTb_d)
            lu_whhT = load("lu_whhT", [128, 4, 128], lu_whhT_d)
            lu_b = load("lu_b", [128, 4], lu_b_d, F32)
            vw0T = load("vw0T", [128, 128], vw0T_d)
            vw1T = load("vw1T", [128, 128], vw1T_d)
            vw2T = load("vw2T", [128, 1], vw2T_d)
            vb = load("vb", [128, 2], vb_d, F32)

            h_lit = load("h_lit", [128, NLIT], h0_lit_d)
            h_cl = load("h_cl", [128, NCL], h0_cl_d)
            c_lit = singles.tile([128, NLIT], F32, tag="c_lit")
            c_cl = singles.tile([128, NCL], F32, tag="c_cl")
            nc.vector.memset(c_lit[:], 0.0)
            nc.vector.memset(c_cl[:], 0.0)

            def mlp_msg(h_t, ncols, col_chunks, wT, b01, nch, m_tag):
                """3-layer MLP feature-major; last layer emitted node-major
                (no bias — folded into the aggregation)."""
                x1 = work.tile([128, NCL], F32R, tag="mx1", bufs=1)
                x2 = work.tile([128, NCL], F32R, tag="mx2", bufs=1)
                for (c0, w) in col_chunks:
                    ps = psp.tile([128, 400], F32, tag="ps")
                    nc.tensor.matmul(ps[:, :w], wT[:, 0, :], h_t[:, c0:c0 + w],
                                     start=True, stop=True)
                    nc.scalar.activation(x1[:, c0:c0 + w], ps[:, :w], AF.Relu,
                                         bias=b01[:, 0:1])
                for (c0, w) in col_chunks:
                    ps = psp.tile([128, 400], F32, tag="ps")
                    nc.tensor.matmul(ps[:, :w], wT[:, 1, :], x1[:, c0:c0 + w],
                                     start=True, stop=True)
                    nc.scalar.activation(x2[:, c0:c0 + w], ps[:, :w], AF.Relu,
                                         bias=b01[:, 1:2])
                m_nm = work.tile([128, nch, 128], F32R, tag=m_tag, bufs=1)
                for i in range(nch):
                    k = 128 if i < nch - 1 else ncols - 128 * (nch - 1)
                    ps = psp.tile([128, 400], F32, tag="ps")
                    nc.tensor.matmul(ps[:k, 0:128], x2[:, 128 * i:128 * i + k],
                                     wT[:, 2, :], start=True, stop=True)
                    nc.vector.tensor_copy(m_nm[:k, i, :], ps[:k, 0:128])
                return m_nm

            for r in range(rounds):
                # ---- forward: literal messages -> clauses ----
                m_nm = mlp_msg(h_lit, NLIT, LIT_COLS, lmsgT, lmsg_b, LCH, "mL")
                agg_c = work.tile([128, NCL], F32R, tag="agg_c", bufs=1)
                for (c0, w) in CL_COLS:
                    ps = psp.tile([128, 400], F32, tag="ps")
                    for i in range(LCH):
                        k = _lchunk(i)
                        nc.tensor.matmul(ps[:, :w], m_nm[:k, i, :],
                                         a_lc[:k, i, c0:c0 + w],
                                         start=(i == 0), stop=(i == LCH - 1))
                    nc.scalar.activation(agg_c[:, c0:c0 + w], ps[:, :w],
                                         AF.Identity, bias=aggc_b[:, 0:1])

                # ---- clause LSTM ----
                for (c0, w) in CL_COLS:
                    gs = []
                    for gi in range(4):
                        ps = psg.tile([128, 400], F32, tag="psg")
                        nc.tensor.matmul(ps[:, :w], cu_wihT[:, gi, :],
                                         agg_c[:, c0:c0 + w],
                                         start=True, stop=False)
                        nc.tensor.matmul(ps[:, :w], cu_whhT[:, gi, :],
                                         h_cl[:, c0:c0 + w],
                                         start=False, stop=True)
                        g_sb = work.tile([128, 400], F32, tag=f"cg{gi}")
                        f = AF.Tanh if gi == 2 else AF.Sigmoid
                        nc.scalar.activation(g_sb[:, :w], ps[:, :w], f,
                                             bias=cu_b[:, gi:gi + 1])
                        gs.append(g_sb)
                    i_, f_, g_, o_ = gs
                    t1 = work.tile([128, 400], F32, tag="t1")
                    t2 = work.tile([128, 400], F32, tag="t2")
                    nc.vector.tensor_mul(t1[:, :w], f_[:, :w], c_cl[:, c0:c0 + w])
                    nc.vector.tensor_mul(t2[:, :w], i_[:, :w], g_[:, :w])
                    nc.vector.tensor_add(c_cl[:, c0:c0 + w], t1[:, :w], t2[:, :w])
                    tc2 = work.tile([128, 400], F32, tag="tc2")
                    nc.scalar.activation(tc2[:, :w], c_cl[:, c0:c0 + w], AF.Tanh)
                    nc.vector.tensor_mul(h_cl[:, c0:c0 + w], o_[:, :w], tc2[:, :w])

                # ---- backward: clause messages -> literals ----
                m2_nm = mlp_msg(h_cl, NCL, CL_COLS, cmsgT, cmsg_b, CCH, "mC")
                agg_l = work.tile([128, NLIT], F32R, tag="agg_l", bufs=1)
                for (c0, w) in LIT_COLS:
                    ps = psp.tile([128, 400], F32, tag="ps")
                    for j in range(CCH):
                        k = _cchunk(j)
                        nc.tensor.matmul(ps[:, :w], m2_nm[:k, j, :],
                                         a_cl[:k, j, c0:c0 + w],
                                         start=(j == 0), stop=(j == CCH - 1))
                    nc.vector.tensor_add(agg_l[:, c0:c0 + w], ps[:, :w],
                                         aggl_b[:, c0:c0 + w])

                # ---- literal LSTM (gates first: reads of old h_lit must
                # precede the in-place h_lit update of either chunk) ----
                lit_gates = {}
                for ci, (c0, w) in enumerate(LIT_COLS):
                    f0 = 400 if c0 == 0 else 0  # flip partner slice
                    for gi in range(4):
                        ps = psg.tile([128, 400], F32, tag="psg")
                        nc.tensor.matmul(ps[:, :w], lu_wihTa[:, gi, :],
                                         agg_l[:, c0:c0 + w],
                                         start=True, stop=False)
                        nc.tensor.matmul(ps[:, :w], lu_wihTb[:, gi, :],
                                         h_lit[:, f0:f0 + w],
                                         start=False, stop=False)
                        nc.tensor.matmul(ps[:, :w], lu_whhT[:, gi, :],
                                         h_lit[:, c0:c0 + w],
                                         start=False, stop=True)
                        g_sb = work.tile([128, 400], F32, tag=f"lg{gi}c{ci}", bufs=1)
                        f = AF.Tanh if gi == 2 else AF.Sigmoid
                        nc.scalar.activation(g_sb[:, :w], ps[:, :w], f,
                                             bias=lu_b[:, gi:gi + 1])
                        lit_gates[(ci, gi)] = g_sb
                for ci, (c0, w) in enumerate(LIT_COLS):
                    i_ = lit_gates[(ci, 0)]
                    f_ = lit_gates[(ci, 1)]
                    g_ = lit_gates[(ci, 2)]
                    o_ = lit_gates[(ci, 3)]
                    t1 = work.tile([128, 400], F32, tag="t1")
                    t2 = work.tile([128, 400], F32, tag="t2")
                    nc.vector.tensor_mul(t1[:, :w], f_[:, :w], c_lit[:, c0:c0 + w])
                    nc.vector.tensor_mul(t2[:, :w], i_[:, :w], g_[:, :w])
                    nc.vector.tensor_add(c_lit[:, c0:c0 + w], t1[:, :w], t2[:, :w])
                    tc2 = work.tile([128, 400], F32, tag="tc2")
                    nc.scalar.activation(tc2[:, :w], c_lit[:, c0:c0 + w], AF.Tanh)
                    nc.vector.tensor_mul(h_lit[:, c0:c0 + w], o_[:, :w], tc2[:, :w])

            # ---- vote head: mean over literals (sum on device) ----
            v1 = work.tile([128, NLIT], F32R, tag="v1", bufs=1)
            v2 = work.tile([128, NLIT], F32R, tag="v2", bufs=1)
            for (c0, w) in LIT_COLS:
                ps = psp.tile([128, 400], F32, tag="ps")
                nc.tensor.matmul(ps[:, :w], vw0T[:], h_lit[:, c0:c0 + w],
                                 start=True, stop=True)
                nc.scalar.activation(v1[:, c0:c0 + w], ps[:, :w], AF.Relu,
                                     bias=vb[:, 0:1])
            for (c0, w) in LIT_COLS:
                ps = psp.tile([128, 400], F32, tag="ps")
                nc.tensor.matmul(ps[:, :w], vw1T[:], v1[:, c0:c0 + w],
                                 start=True, stop=True)
                nc.scalar.activation(v2[:, c0:c0 + w], ps[:, :w], AF.Relu,
                                     bias=vb[:, 1:2])
            acc = work.tile([1, 2], F32, tag="acc")
            for ci, (c0, w) in enumerate(LIT_COLS):
                ps = psp.tile([1, 400], F32, tag="psv", bufs=1)
                nc.tensor.matmul(ps[:, :w], vw2T[:], v2[:, c0:c0 + w],
                                 start=True, stop=True)
                nc.vector.reduce_sum(acc[:, ci:ci + 1], ps[:, :w],
                                     axis=mybir.AxisListType.X)
            total = work.tile([1, 1], F32, tag="total")
            nc.vector.tensor_add(total[:], acc[:, 0:1], acc[:, 1:2])
            nc.sync.dma_start(out=out_d[:], in_=total[:])

            if debug_state:
                hl32 = work.tile([128, NLIT], F32, tag="dbg1")
                nc.vector.tensor_copy(hl32[:], h_lit[:])
                nc.sync.dma_start(out=dbg_hl[:], in_=hl32[:])
                hc32 = work.tile([128, NCL], F32, tag="dbg2")
                nc.vector.tensor_copy(hc32[:], h_cl[:])
                nc.sync.dma_start(out=dbg_hc[:], in_=hc32[:])
                nc.sync.dma_start(out=dbg_cl[:], in_=c_lit[:])
                nc.sync.dma_start(out=dbg_cc[:], in_=c_cl[:])

    nc.compile()
    return nc


def prep_inputs(inputs):
    """Host-side prep: per-core input dicts from the full problem inputs."""
    f32 = np.float32
    edge_src = np.asarray(inputs["edge_src"]).reshape(NG, NCL * KLIT)
    edge_dst = np.asarray(inputs["edge_dst"]).reshape(NG, NCL * KLIT)

    lmsg_w = np.asarray(inputs["lmsg_w"], f32)
    lmsg_b = np.asarray(inputs["lmsg_b"], f32)
    cmsg_w = np.asarray(inputs["cmsg_w"], f32)
    cmsg_b = np.asarray(inputs["cmsg_b"], f32)

    lmsgT = np.ascontiguousarray(np.transpose(lmsg_w, (2, 0, 1)))  # [128,3,128]
    cmsgT = np.ascontiguousarray(np.transpose(cmsg_w, (2, 0, 1)))
    lmsg_b01 = np.ascontiguousarray(lmsg_b[0:2].T)  # [128,2]
    cmsg_b01 = np.ascontiguousarray(cmsg_b[0:2].T)
    aggc_b = np.ascontiguousarray((KLIT * lmsg_b[2])[:, None])  # [128,1]

    def gate_pack(w):  # [512, din] -> [din, 4, 128]
        return np.ascontiguousarray(
            np.transpose(w.reshape(4, 128, -1), (2, 0, 1)))

    cu_wihT = gate_pack(np.asarray(inputs["cu_wih"], f32))
    cu_whhT = gate_pack(np.asarray(inputs["cu_whh"], f32))
    cu_b = np.ascontiguousarray(
        (np.asarray(inputs["cu_bih"], f32)
         + np.asarray(inputs["cu_bhh"], f32)).reshape(4, 128).T)
    lu_wih = np.asarray(inputs["lu_wih"], f32)  # [512, 256]
    lu_wihTa = gate_pack(lu_wih[:, :128])
    lu_wihTb = gate_pack(lu_wih[:, 128:])
    lu_whhT = gate_pack(np.asarray(inputs["lu_whh"], f32))
    lu_b = np.ascontiguousarray(
        (np.asarray(inputs["lu_bih"], f32)
         + np.asarray(inputs["lu_bhh"], f32)).reshape(4, 128).T)

    vw0T = np.ascontiguousarray(np.asarray(inputs["vote_w0"], f32).T)
    vw1T = np.ascontiguousarray(np.asarray(inputs["vote_w1"], f32).T)
    vw2T = np.ascontiguousarray(np.asarray(inputs["vote_w2"], f32).T)  # [128,1]
    vb = np.stack([np.asarray(inputs["vote_b0"], f32),
                   np.asarray(inputs["vote_b1"], f32)], axis=1)  # [128,2]

    h0l = (np.asarray(inputs["L_init_w"], f32)[:, 0]
           + np.asarray(inputs["L_init_b"], f32))  # [128]
    h0c = (np.asarray(inputs["C_init_w"], f32)[:, 0]
           + np.asarray(inputs["C_init_b"], f32))
    h0_lit = np.ascontiguousarray(np.broadcast_to(h0l[:, None], (128, NLIT)))
    h0_cl = np.ascontiguousarray(np.broadcast_to(h0c[:, None], (128, NCL)))

    cmsg_b2 = cmsg_b[2]  # [128]

    in_maps = []
    for g in range(NG):
        src = edge_src[g] - g * NNG          # local literal ids [0, 800)
        dst = edge_dst[g] - g * NNG - NLIT   # local clause ids [0, 1200)
        A = np.zeros((LCH * 128, NCL), f32)
        np.add.at(A, (src, dst), 1.0)
        deg = A.sum(axis=1)[:NLIT]           # literal degrees
        a_lc = np.ascontiguousarray(
            A.reshape(LCH, 128, NCL).transpose(1, 0, 2))
        At = np.zeros((CCH * 128, NLIT), f32)
        At[:NCL] = A[:NLIT].T
        a_cl = np.ascontiguousarray(
            At.reshape(CCH, 128, NLIT).transpose(1, 0, 2))
        aggl_b = np.ascontiguousarray(np.outer(cmsg_b2, deg))  # [128,800]

        in_maps.append(dict(
            a_lc=a_lc, a_cl=a_cl, h0_lit=h0_lit, h0_cl=h0_cl,
            lmsgT=lmsgT, lmsg_b=lmsg_b01, cmsgT=cmsgT, cmsg_b=cmsg_b01,
            aggc_b=aggc_b, aggl_b=aggl_b,
            cu_wihT=cu_wihT, cu_whhT=cu_whhT, cu_b=cu_b,
            lu_wihTa=lu_wihTa, lu_wihTb=lu_wihTb, lu_whhT=lu_whhT, lu_b=lu_b,
            vw0T=vw0T, vw1T=vw1T, vw2T=vw2T, vb=vb,
        ))
    return in_maps


_NC_CACHE = {}
LAST_RESULT = None


def kernel(**inputs):
    global LAST_RESULT
    key = "main"
    if key not in _NC_CACHE:
        _NC_CACHE[key] = build_nc()
    nc = _NC_CACHE[key]
    in_maps = prep_inputs(inputs)
    res = run_bass_kernel_spmd(nc, in_maps, list(range(NG)))
    LAST_RESULT = res
    vote_b2 = float(np.asarray(inputs["vote_b2"], np.float32)[0])
    n_vars = np.asarray(inputs["n_vars"]).astype(np.float32)
    sums = np.array([res.results[g]["out"][0, 0] for g in range(NG)],
                    np.float32)
    sums = sums + np.float32(NLIT * vote_b2)
    return (sums / (2.0 * n_vars)).astype(np.float32)
